# revision 5
# baseline (speedup 1.0000x reference)
"""AttentionMIL Trainium2 kernel (v2: fp8 DoubleRow encoder).

Math (per bag of 512 instances):
    emb    = relu(x @ w_enc + b_enc)            [512, 128]
    a      = tanh(emb @ w_att + b_att)          [512, 64]
    logits = a @ w_score (+ b_score, dropped: softmax shift-invariant)
    attn   = softmax(logits) within the bag
    bag    = sum_i attn[i] * emb[i]             [128]
    score  = bag @ w_cls + b_cls                [2]

Distribution: data-parallel over bags. 8 NeuronCores, 8 bags (4096
instances) per core, weights replicated, no cross-core communication.
Each core returns its 8 bags' scores transposed [2, 8]; host stacks.

v2 changes vs the bf16 baseline (52 us):
 - x and w_enc are fp8 e4m3 (TRN FP8_EXP4 == ml_dtypes.float8_e4m3,
   max 240). Halves the dominant HBM stream (8.4 -> 4.2 MB/core) and
   enables DoubleRow matmuls (2 fp8 weights/PE cell, K=256 per pass):
   the 8 K=128 bf16 encoder matmuls per bag become 4 K=256 fp8 ones.
   w_enc is pre-scaled by 64 on the host so its ~N(0, 1/1024) entries
   sit in fp8's normal range; the relu ACT un-scales via scale=1/64.
   Numpy-validated rel err ~5.7e-3 vs the f32 reference (gate 2e-2).
 - x is host-packed bag-major [bag, 128, chunk, inst] so each per-bag
   slab DMA reads 4 KB contiguous per partition (was 2x1KB pieces).
 - All 8 slab DMAs issue up front on the sync queue (8 distinct
   buffers, no reuse stalls); weights are packed into 3 DMAs (fp8
   w_enc / bf16 w_att|w_score|w_cls / f32 biases) on the scalar queue
   so first matmul starts ~4 us earlier. Bag 0's slab is split in two
   chunk-halves so its first DoubleRow pair starts sooner.
 - Softmax tail uses fused accumulate outputs: the Exp ACT emits the
   denominator via accum_out, and one DVE tensor_tensor_reduce does
   the e-weighted bag reduction (was mul + reduce). The e-broadcast
   stays a K=1 matmul.
 - Classifier epilogue: normalize bag columns once (reciprocal ->
   K=1 f32r broadcast -> DVE mul to bf16), one bf16 cls matmul, one
   bias ACT, one out DMA.
"""

import sys

sys.path.insert(0, "/opt/trn_rl_repo")

import numpy as np

N_INST = 32768
N_BAGS = 64
D_IN = 1024
D_EMB = 128
D_ATT = 64
N_CLS = 2

N_CORES = 8
BAGS_PER_CORE = N_BAGS // N_CORES          # 8
INST_PER_BAG = N_INST // N_BAGS            # 512
INST_PER_CORE = N_INST // N_CORES          # 4096
DIN_CHUNKS = D_IN // 128                   # 8
DR_PAIRS = DIN_CHUNKS // 2                 # 4 DoubleRow K=256 passes
W_SCALE = 64.0                             # host pre-scale on w_enc

# fused accumulate outputs (ACT accum_out / DVE tensor_tensor_reduce);
# fall back to explicit DVE reduce_sum when False
USE_ACT_ACCUM = True
USE_TTR = False  # InstTensorTensorReduce dies at NEFF runtime (probe c2)

_CACHE = {}


def _build():
    import concourse.bacc as bacc
    import concourse.mybir as mybir
    import concourse.tile as tile

    f32 = mybir.dt.float32
    f32r = mybir.dt.float32r
    bf16 = mybir.dt.bfloat16
    fp8 = mybir.dt.float8e4
    AF = mybir.ActivationFunctionType
    DR = mybir.MatmulPerfMode.DoubleRow
    MUL = mybir.AluOpType.mult
    ADD = mybir.AluOpType.add

    nc = bacc.Bacc("TRN2", target_bir_lowering=False, debug=False,
                   enable_asserts=False, num_devices=N_CORES)

    xt = nc.dram_tensor("xt", [BAGS_PER_CORE, 128, DIN_CHUNKS, INST_PER_BAG],
                        fp8, kind="ExternalInput")
    wq = nc.dram_tensor("wq", [128, DIN_CHUNKS, D_EMB], fp8,
                        kind="ExternalInput")
    wb = nc.dram_tensor("wb", [128, D_ATT + 1 + N_CLS], bf16,
                        kind="ExternalInput")
    wf = nc.dram_tensor("wf", [128, 3], f32, kind="ExternalInput")
    out = nc.dram_tensor("out", [N_CLS, BAGS_PER_CORE], f32,
                         kind="ExternalOutput")

    with tile.TileContext(nc) as tc:
        with (
            tc.tile_pool(name="const", bufs=1) as const,
            tc.tile_pool(name="xt", bufs=1) as xt_pool,
            tc.tile_pool(name="work", bufs=3) as work,
            tc.tile_pool(name="ps", bufs=2, space="PSUM") as ps,
        ):
            # ---- x slab DMAs first: the stream is the critical path ----
            # bag 0 split into chunk-halves so compute starts sooner
            s0a = xt_pool.tile([128, DR_PAIRS // 2 * 2, INST_PER_BAG], fp8,
                               tag="s0a")
            nc.sync.dma_start(out=s0a, in_=xt[0, :, 0:4, :])
            s0b = xt_pool.tile([128, DR_PAIRS // 2 * 2, INST_PER_BAG], fp8,
                               tag="s0b")
            nc.sync.dma_start(out=s0b, in_=xt[0, :, 4:8, :])
            slabs = [None]
            for b in range(1, BAGS_PER_CORE):
                t = xt_pool.tile([128, DIN_CHUNKS, INST_PER_BAG], fp8,
                                 tag="slab", bufs=BAGS_PER_CORE - 1)
                nc.sync.dma_start(out=t, in_=xt[b])
                slabs.append(t)

            # ---- replicated weights, packed, on the scalar queue ----
            wq_sb = const.tile([128, DIN_CHUNKS, D_EMB], fp8)
            nc.scalar.dma_start(out=wq_sb, in_=wq[:, :, :])
            wb_sb = const.tile([128, D_ATT + 1 + N_CLS], bf16)
            nc.scalar.dma_start(out=wb_sb, in_=wb[:, :])
            wf_sb = const.tile([128, 3], f32)
            nc.scalar.dma_start(out=wf_sb, in_=wf[:, :])

            watt = wb_sb[:, 0:D_ATT]                 # [128, 64] bf16
            wscore = wb_sb[0:D_ATT, D_ATT:D_ATT + 1]  # [64, 1] bf16
            wcls = wb_sb[:, D_ATT + 1:D_ATT + 3]     # [128, 2] bf16
            benc = wf_sb[:, 0:1]                     # [128, 1] f32
            batt = wf_sb[0:D_ATT, 1:2]               # [64, 1] f32
            bcls = wf_sb[0:N_CLS, 2:3]               # [2, 1] f32

            ones_row = const.tile([1, 128], bf16)
            nc.vector.memset(ones_row, 1.0)
            ones_f32r = const.tile([1, 128], f32r)
            ones_tmp = const.tile([1, 128], f32)
            nc.vector.memset(ones_tmp, 1.0)
            nc.vector.tensor_copy(ones_f32r, ones_tmp)

            # unnormalized bag embeddings (columns) + softmax denominators
            bag_all = const.tile([D_EMB, BAGS_PER_CORE], f32)
            den_all = const.tile([1, BAGS_PER_CORE], f32)

            def emit_enc(b):
                # emb^T = relu((sum over 4 DoubleRow K=256 passes) / 64 + b)
                ps_emb = ps.tile([D_EMB, INST_PER_BAG], f32, tag="emb")
                for c in range(DR_PAIRS):
                    if b == 0:
                        src = s0a if c < 2 else s0b
                        mv = src[:, 2 * (c % 2):2 * (c % 2) + 2, :]
                    else:
                        mv = slabs[b][:, 2 * c:2 * c + 2, :]
                    nc.tensor.matmul(ps_emb[:, :], wq_sb[:, 2 * c:2 * c + 2, :],
                                     mv, start=(c == 0), stop=(c == DR_PAIRS - 1),
                                     perf_mode=DR)
                embT = work.tile([D_EMB, INST_PER_BAG], bf16, tag="embT")
                nc.scalar.activation(embT, ps_emb, AF.Relu, bias=benc,
                                     scale=1.0 / W_SCALE)
                return embT

            def emit_tail(b, embT, i0, n, den_col, bag_col):
                # softmax tail for instances [i0, i0+n) of bag b
                sl = slice(i0, i0 + n)
                ps_a = ps.tile([D_ATT, n], f32, tag="a")
                nc.tensor.matmul(ps_a[:, :], watt, embT[:, sl],
                                 start=True, stop=True)
                aT = work.tile([D_ATT, n], bf16, tag="aT")
                nc.scalar.activation(aT, ps_a, AF.Tanh, bias=batt, scale=1.0)

                ps_l = ps.tile([1, n], f32, tag="logit")
                nc.tensor.matmul(ps_l[:, :], wscore, aT[:, :],
                                 start=True, stop=True)
                # softmax numerator + denominator in one ACT (no max-shift:
                # logits = a @ w_score with a in (-1,1), |logits| <~ 6)
                e_row = work.tile([1, n], bf16, tag="e_row")
                if USE_ACT_ACCUM:
                    nc.scalar.activation(e_row, ps_l, AF.Exp, scale=1.0,
                                         accum_out=den_col)
                else:
                    nc.scalar.activation(e_row, ps_l, AF.Exp, scale=1.0)
                    nc.vector.reduce_sum(den_col, e_row,
                                         axis=mybir.AxisListType.X)
                # broadcast e across partitions via K=1 matmul, then one
                # fused DVE mul+reduce for the unnormalized bag column
                ps_bc = ps.tile([D_EMB, n], f32, tag="bc")
                nc.tensor.matmul(ps_bc[:, :], ones_row[:, :], e_row[:, :],
                                 start=True, stop=True)
                scratch = work.tile([D_EMB, n], bf16, tag="scratch")
                if USE_TTR:
                    nc.vector.tensor_tensor_reduce(
                        out=scratch, in0=embT[:, sl], in1=ps_bc[:, :],
                        scale=1.0, scalar=0.0, op0=MUL, op1=ADD,
                        accum_out=bag_col)
                else:
                    nc.vector.tensor_mul(scratch, embT[:, sl], ps_bc[:, :])
                    nc.vector.reduce_sum(bag_col, scratch,
                                         axis=mybir.AxisListType.X)

            # software pipeline: emit bag b's dependent tail after bag b+1's
            # encoder matmuls so the in-order PE queue never head-of-line
            # blocks on the softmax chain
            prev = None
            for b in range(BAGS_PER_CORE):
                embT = emit_enc(b)
                if prev is not None:
                    emit_tail(b - 1, prev, 0, INST_PER_BAG,
                              den_all[0:1, b - 1:b], bag_all[:, b - 1:b])
                prev = embT

            # the last bag's tail is the serial end-of-kernel chain: split
            # into two halves so PE/ACT/DVE stages pipeline
            H = INST_PER_BAG // 2
            den_h = work.tile([1, 2], f32, tag="den_h")
            bag_h = work.tile([D_EMB, 2], f32, tag="bag_h")
            bL = BAGS_PER_CORE - 1
            for h in range(2):
                emit_tail(bL, prev, h * H, H,
                          den_h[0:1, h:h + 1], bag_h[:, h:h + 1])
            nc.vector.tensor_add(den_all[0:1, bL:bL + 1], den_h[0:1, 0:1],
                                 den_h[0:1, 1:2])
            nc.vector.tensor_add(bag_all[:, bL:bL + 1], bag_h[:, 0:1],
                                 bag_h[:, 1:2])

            # ---- classifier epilogue ----
            # normalize the bag columns, then scores = w_cls.T @ bag + b_cls
            rden = const.tile([1, BAGS_PER_CORE], f32r)
            with nc.allow_low_precision(reason="1/denom at f32r, ~1e-4 rel"):
                nc.vector.reciprocal(rden, den_all)
            ps_r = ps.tile([D_EMB, BAGS_PER_CORE], f32, tag="bc")
            nc.tensor.matmul(ps_r[:, :], ones_f32r[:, :], rden[:, :],
                             start=True, stop=True)
            bag_n = const.tile([D_EMB, BAGS_PER_CORE], bf16)
            nc.vector.tensor_mul(bag_n, bag_all, ps_r[:, :])
            ps_s = ps.tile([N_CLS, BAGS_PER_CORE], f32, tag="logit")
            nc.tensor.matmul(ps_s[:, :], wcls, bag_n[:, :],
                             start=True, stop=True)
            scores = const.tile([N_CLS, BAGS_PER_CORE], f32)
            nc.scalar.activation(scores, ps_s, AF.Identity, bias=bcls,
                                 scale=1.0)
            nc.scalar.dma_start(out=out[:, :], in_=scores)

    nc.compile()
    return nc


def _pack_inputs(x, w_enc, b_enc, w_att, b_att, w_score, w_cls, b_cls):
    """Host-side packing: per-core input maps for run_bass_kernel_spmd."""
    import ml_dtypes

    f8 = ml_dtypes.float8_e4m3
    bf16 = ml_dtypes.bfloat16

    # x [N_INST, D_IN] -> [core, bag, p, chunk, inst] fp8
    xp = x.reshape(N_CORES, BAGS_PER_CORE, INST_PER_BAG, DIN_CHUNKS, 128)
    xp = np.ascontiguousarray(xp.transpose(0, 1, 4, 3, 2)).astype(f8)

    # w_enc [D_IN, D_EMB] * 64 -> [p, chunk, emb] fp8
    wq = (w_enc * W_SCALE).reshape(DIN_CHUNKS, 128, D_EMB).transpose(1, 0, 2)
    wq = np.ascontiguousarray(wq).astype(f8)

    # bf16 pack: w_att | w_score (col, zero-padded) | w_cls
    wscore_col = np.zeros((D_EMB, 1), dtype=np.float32)
    wscore_col[:D_ATT, 0] = w_score
    wbp = np.concatenate(
        [w_att, wscore_col, w_cls], axis=1).astype(bf16)

    # f32 pack: b_enc | b_att (padded) | b_cls (padded)
    wfp = np.zeros((D_EMB, 3), dtype=np.float32)
    wfp[:, 0] = b_enc
    wfp[:D_ATT, 1] = b_att
    wfp[:N_CLS, 2] = b_cls

    shared = {"wq": wq, "wb": wbp, "wf": wfp}
    return [{"xt": xp[c], **shared} for c in range(N_CORES)]


def _numpy_fallback(x, seg, w_enc, b_enc, w_att, b_att, w_score, b_score,
                    w_cls, b_cls):
    emb = np.maximum(x @ w_enc + b_enc, 0.0)
    a = np.tanh(emb @ w_att + b_att)
    logits = a @ w_score + b_score[0]
    out = np.zeros((N_BAGS, N_CLS), dtype=np.float32)
    for bag in range(N_BAGS):
        mask = seg == bag
        lg = logits[mask]
        e = np.exp(lg - lg.max())
        attn = e / e.sum()
        bag_emb = attn @ emb[mask]
        out[bag] = bag_emb @ w_cls + b_cls
    return out


def kernel(**inputs):
    from concourse.bass_utils import run_bass_kernel_spmd

    x = np.asarray(inputs["x"], dtype=np.float32)
    seg = np.asarray(inputs["seg"], dtype=np.int32)
    w_enc = np.asarray(inputs["w_enc"], dtype=np.float32)
    b_enc = np.asarray(inputs["b_enc"], dtype=np.float32)
    w_att = np.asarray(inputs["w_att"], dtype=np.float32)
    b_att = np.asarray(inputs["b_att"], dtype=np.float32)
    w_score = np.asarray(inputs["w_score"], dtype=np.float32)
    b_score = np.asarray(inputs["b_score"], dtype=np.float32)
    w_cls = np.asarray(inputs["w_cls"], dtype=np.float32)
    b_cls = np.asarray(inputs["b_cls"], dtype=np.float32)

    expected_seg = np.repeat(np.arange(N_BAGS, dtype=np.int32), INST_PER_BAG)
    if not np.array_equal(seg, expected_seg):
        # Layout differs from the balanced bags this kernel is built for.
        return _numpy_fallback(x, seg, w_enc, b_enc, w_att, b_att, w_score,
                               b_score, w_cls, b_cls)

    if "nc" not in _CACHE:
        _CACHE["nc"] = _build()
    nc = _CACHE["nc"]

    in_maps = _pack_inputs(x, w_enc, b_enc, w_att, b_att, w_score, w_cls,
                           b_cls)
    res = run_bass_kernel_spmd(nc, in_maps, core_ids=list(range(N_CORES)))
    return np.concatenate(
        [res.results[c]["out"].T for c in range(N_CORES)], axis=0)


# revision 8
# speedup vs baseline: 1.0622x; 1.0622x over previous
"""AttentionMIL Trainium2 kernel (v2: fp8 DoubleRow encoder).

Math (per bag of 512 instances):
    emb    = relu(x @ w_enc + b_enc)            [512, 128]
    a      = tanh(emb @ w_att + b_att)          [512, 64]
    logits = a @ w_score (+ b_score, dropped: softmax shift-invariant)
    attn   = softmax(logits) within the bag
    bag    = sum_i attn[i] * emb[i]             [128]
    score  = bag @ w_cls + b_cls                [2]

Distribution: data-parallel over bags. 8 NeuronCores, 8 bags (4096
instances) per core, weights replicated, no cross-core communication.
Each core returns its 8 bags' scores transposed [2, 8]; host stacks.

v2 changes vs the bf16 baseline (52 us):
 - x and w_enc are fp8 e4m3 (TRN FP8_EXP4 == ml_dtypes.float8_e4m3,
   max 240). Halves the dominant HBM stream (8.4 -> 4.2 MB/core) and
   enables DoubleRow matmuls (2 fp8 weights/PE cell, K=256 per pass):
   the 8 K=128 bf16 encoder matmuls per bag become 4 K=256 fp8 ones.
   w_enc is pre-scaled by 64 on the host so its ~N(0, 1/1024) entries
   sit in fp8's normal range; the relu ACT un-scales via scale=1/64.
   Numpy-validated rel err ~5.7e-3 vs the f32 reference (gate 2e-2).
 - x is host-packed bag-major [bag, 128, chunk, inst] so each per-bag
   slab DMA reads 4 KB contiguous per partition (was 2x1KB pieces).
 - All 8 slab DMAs issue up front on the sync queue (8 distinct
   buffers, no reuse stalls); weights are packed into 3 DMAs (fp8
   w_enc / bf16 w_att|w_score|w_cls / f32 biases) on the scalar queue
   so first matmul starts ~4 us earlier. Bag 0's slab is split in two
   chunk-halves so its first DoubleRow pair starts sooner.
 - Softmax tail uses fused accumulate outputs: the Exp ACT emits the
   denominator via accum_out, and one DVE tensor_tensor_reduce does
   the e-weighted bag reduction (was mul + reduce). The e-broadcast
   stays a K=1 matmul.
 - Classifier epilogue: normalize bag columns once (reciprocal ->
   K=1 f32r broadcast -> DVE mul to bf16), one bf16 cls matmul, one
   bias ACT, one out DMA.
"""

import sys

sys.path.insert(0, "/opt/trn_rl_repo")

import numpy as np

N_INST = 32768
N_BAGS = 64
D_IN = 1024
D_EMB = 128
D_ATT = 64
N_CLS = 2

N_CORES = 8
BAGS_PER_CORE = N_BAGS // N_CORES          # 8
INST_PER_BAG = N_INST // N_BAGS            # 512
INST_PER_CORE = N_INST // N_CORES          # 4096
DIN_CHUNKS = D_IN // 128                   # 8
DR_PAIRS = DIN_CHUNKS // 2                 # 4 DoubleRow K=256 passes
W_SCALE = 64.0                             # host pre-scale on w_enc

# fused accumulate outputs (ACT accum_out / DVE tensor_tensor_reduce);
# fall back to explicit DVE reduce_sum when False
USE_ACT_ACCUM = True
USE_TTR = False  # InstTensorTensorReduce dies at NEFF runtime (probe c2)
# DoubleRow measured ~512 ns warm per K=256/N=512 pass vs 2x215 ns for two
# plain K=128 passes — no win with the split-pair layout; plain fp8 runs at
# bf16 speed and keeps the halved DMA stream.
USE_DR = False
WARMUP_MMS = 10  # back-to-back dummy matmuls to flip the PE HAM clock gate

_CACHE = {}


def _build():
    import concourse.bacc as bacc
    import concourse.mybir as mybir
    import concourse.tile as tile

    f32 = mybir.dt.float32
    f32r = mybir.dt.float32r
    bf16 = mybir.dt.bfloat16
    fp8 = mybir.dt.float8e4
    AF = mybir.ActivationFunctionType
    DR = mybir.MatmulPerfMode.DoubleRow
    MUL = mybir.AluOpType.mult
    ADD = mybir.AluOpType.add

    nc = bacc.Bacc("TRN2", target_bir_lowering=False, debug=False,
                   enable_asserts=False, num_devices=N_CORES)

    xt = nc.dram_tensor("xt", [BAGS_PER_CORE, 128, DIN_CHUNKS, INST_PER_BAG],
                        fp8, kind="ExternalInput")
    wq = nc.dram_tensor("wq", [128, DIN_CHUNKS, D_EMB], fp8,
                        kind="ExternalInput")
    wb = nc.dram_tensor("wb", [128, D_ATT + 1 + N_CLS], bf16,
                        kind="ExternalInput")
    wf = nc.dram_tensor("wf", [128, 3], f32, kind="ExternalInput")
    out = nc.dram_tensor("out", [N_CLS, BAGS_PER_CORE], f32,
                         kind="ExternalOutput")

    with tile.TileContext(nc) as tc:
        with (
            tc.tile_pool(name="const", bufs=1) as const,
            tc.tile_pool(name="xt", bufs=1) as xt_pool,
            tc.tile_pool(name="work", bufs=3) as work,
            tc.tile_pool(name="ps", bufs=2, space="PSUM") as ps,
        ):
            # ---- sync queue: encoder weights, then the x slab stream ----
            # wq first — it gates the first matmul (scalar queue lags)
            wq_sb = const.tile([128, DIN_CHUNKS, D_EMB], fp8)
            nc.sync.dma_start(out=wq_sb, in_=wq[:, :, :])
            # bag 0 split into chunk-halves so compute starts sooner
            s0a = xt_pool.tile([128, DIN_CHUNKS // 2, INST_PER_BAG], fp8,
                               tag="s0a")
            nc.sync.dma_start(out=s0a, in_=xt[0, :, 0:4, :])
            s0b = xt_pool.tile([128, DIN_CHUNKS // 2, INST_PER_BAG], fp8,
                               tag="s0b")
            nc.sync.dma_start(out=s0b, in_=xt[0, :, 4:8, :])
            slabs = [None]
            for b in range(1, BAGS_PER_CORE):
                t = xt_pool.tile([128, DIN_CHUNKS, INST_PER_BAG], fp8,
                                 tag="slab", bufs=BAGS_PER_CORE - 1)
                nc.sync.dma_start(out=t, in_=xt[b])
                slabs.append(t)

            # ---- remaining weights on the scalar queue ----
            wb_sb = const.tile([128, D_ATT + 1 + N_CLS], bf16)
            nc.scalar.dma_start(out=wb_sb, in_=wb[:, :])
            wf_sb = const.tile([128, 3], f32)
            nc.scalar.dma_start(out=wf_sb, in_=wf[:, :])

            watt = wb_sb[:, 0:D_ATT]                 # [128, 64] bf16
            wscore = wb_sb[0:D_ATT, D_ATT:D_ATT + 1]  # [64, 1] bf16
            wcls = wb_sb[:, D_ATT + 1:D_ATT + 3]     # [128, 2] bf16
            benc = wf_sb[:, 0:1]                     # [128, 1] f32
            batt = wf_sb[0:D_ATT, 1:2]               # [64, 1] f32
            bcls = wf_sb[0:N_CLS, 2:3]               # [2, 1] f32

            ones_row = const.tile([1, 128], bf16)
            nc.vector.memset(ones_row, 1.0)
            ones_f32r = const.tile([1, 128], f32r)
            ones_tmp = const.tile([1, 128], f32)
            nc.vector.memset(ones_tmp, 1.0)
            nc.vector.tensor_copy(ones_f32r, ones_tmp)

            # unnormalized bag embeddings (columns) + softmax denominators
            bag_all = const.tile([D_EMB, BAGS_PER_CORE], f32)
            den_all = const.tile([1, BAGS_PER_CORE], f32)

            # ---- PE warm-up: the HAM clock gate keeps the PE at 1.2 GHz
            # until it has seen ~3.4 us of sustained activity, and on some
            # cores the real matmul stream never flips it (measured: 3/8
            # cores stuck cold the whole kernel). Burn the DMA-wait window
            # on dummy back-to-back matmuls so every core enters the real
            # stream at 2.4 GHz.
            warm_src = work.tile([D_EMB, INST_PER_BAG], bf16, tag="embT")
            nc.vector.memset(warm_src, 0.0)
            ps_warm = ps.tile([D_EMB, INST_PER_BAG], f32, tag="emb")
            for _ in range(WARMUP_MMS):
                nc.tensor.matmul(ps_warm[:, :], warm_src[:, 0:D_EMB],
                                 warm_src[:, :], start=True, stop=True)

            def emit_enc(b):
                # emb^T = relu((x @ w_enc*64) / 64 + b_enc), K chunked by 128
                ps_emb = ps.tile([D_EMB, INST_PER_BAG], f32, tag="emb")
                if USE_DR:
                    for c in range(DR_PAIRS):
                        if b == 0:
                            src = s0a if c < 2 else s0b
                            mv = src[:, 2 * (c % 2):2 * (c % 2) + 2, :]
                        else:
                            mv = slabs[b][:, 2 * c:2 * c + 2, :]
                        nc.tensor.matmul(ps_emb[:, :],
                                         wq_sb[:, 2 * c:2 * c + 2, :], mv,
                                         start=(c == 0),
                                         stop=(c == DR_PAIRS - 1),
                                         perf_mode=DR)
                else:
                    for c in range(DIN_CHUNKS):
                        if b == 0:
                            src = s0a if c < 4 else s0b
                            mv = src[:, c % 4, :]
                        else:
                            mv = slabs[b][:, c, :]
                        nc.tensor.matmul(ps_emb[:, :], wq_sb[:, c, :], mv,
                                         start=(c == 0),
                                         stop=(c == DIN_CHUNKS - 1))
                embT = work.tile([D_EMB, INST_PER_BAG], bf16, tag="embT")
                nc.scalar.activation(embT, ps_emb, AF.Relu, bias=benc,
                                     scale=1.0 / W_SCALE)
                return embT

            def emit_tail(b, embT, i0, n, den_col, bag_col):
                # softmax tail for instances [i0, i0+n) of bag b
                sl = slice(i0, i0 + n)
                ps_a = ps.tile([D_ATT, n], f32, tag="a")
                nc.tensor.matmul(ps_a[:, :], watt, embT[:, sl],
                                 start=True, stop=True)
                aT = work.tile([D_ATT, n], bf16, tag="aT")
                nc.scalar.activation(aT, ps_a, AF.Tanh, bias=batt, scale=1.0)

                ps_l = ps.tile([1, n], f32, tag="logit")
                nc.tensor.matmul(ps_l[:, :], wscore, aT[:, :],
                                 start=True, stop=True)
                # softmax numerator + denominator in one ACT (no max-shift:
                # logits = a @ w_score with a in (-1,1), |logits| <~ 6)
                e_row = work.tile([1, n], bf16, tag="e_row")
                if USE_ACT_ACCUM:
                    nc.scalar.activation(e_row, ps_l, AF.Exp, scale=1.0,
                                         accum_out=den_col)
                else:
                    nc.scalar.activation(e_row, ps_l, AF.Exp, scale=1.0)
                    nc.vector.reduce_sum(den_col, e_row,
                                         axis=mybir.AxisListType.X)
                # broadcast e across partitions via K=1 matmul, then one
                # fused DVE mul+reduce for the unnormalized bag column
                ps_bc = ps.tile([D_EMB, n], f32, tag="bc")
                nc.tensor.matmul(ps_bc[:, :], ones_row[:, :], e_row[:, :],
                                 start=True, stop=True)
                scratch = work.tile([D_EMB, n], bf16, tag="scratch")
                if USE_TTR:
                    nc.vector.tensor_tensor_reduce(
                        out=scratch, in0=embT[:, sl], in1=ps_bc[:, :],
                        scale=1.0, scalar=0.0, op0=MUL, op1=ADD,
                        accum_out=bag_col)
                else:
                    nc.vector.tensor_mul(scratch, embT[:, sl], ps_bc[:, :])
                    nc.vector.reduce_sum(bag_col, scratch,
                                         axis=mybir.AxisListType.X)

            # software pipeline: emit bag b's dependent tail after bag b+1's
            # encoder matmuls so the in-order PE queue never head-of-line
            # blocks on the softmax chain
            prev = None
            for b in range(BAGS_PER_CORE):
                embT = emit_enc(b)
                if prev is not None:
                    emit_tail(b - 1, prev, 0, INST_PER_BAG,
                              den_all[0:1, b - 1:b], bag_all[:, b - 1:b])
                prev = embT

            # the last bag's tail is the serial end-of-kernel chain: split
            # into two halves so PE/ACT/DVE stages pipeline
            H = INST_PER_BAG // 2
            den_h = work.tile([1, 2], f32, tag="den_h")
            bag_h = work.tile([D_EMB, 2], f32, tag="bag_h")
            bL = BAGS_PER_CORE - 1
            for h in range(2):
                emit_tail(bL, prev, h * H, H,
                          den_h[0:1, h:h + 1], bag_h[:, h:h + 1])
            nc.vector.tensor_add(den_all[0:1, bL:bL + 1], den_h[0:1, 0:1],
                                 den_h[0:1, 1:2])
            nc.vector.tensor_add(bag_all[:, bL:bL + 1], bag_h[:, 0:1],
                                 bag_h[:, 1:2])

            # ---- classifier epilogue ----
            # normalize the bag columns, then scores = w_cls.T @ bag + b_cls
            rden = const.tile([1, BAGS_PER_CORE], f32r)
            with nc.allow_low_precision(reason="1/denom at f32r, ~1e-4 rel"):
                nc.vector.reciprocal(rden, den_all)
            ps_r = ps.tile([D_EMB, BAGS_PER_CORE], f32, tag="bc")
            nc.tensor.matmul(ps_r[:, :], ones_f32r[:, :], rden[:, :],
                             start=True, stop=True)
            bag_n = const.tile([D_EMB, BAGS_PER_CORE], bf16)
            nc.vector.tensor_mul(bag_n, bag_all, ps_r[:, :])
            ps_s = ps.tile([N_CLS, BAGS_PER_CORE], f32, tag="logit")
            nc.tensor.matmul(ps_s[:, :], wcls, bag_n[:, :],
                             start=True, stop=True)
            scores = const.tile([N_CLS, BAGS_PER_CORE], f32)
            nc.scalar.activation(scores, ps_s, AF.Identity, bias=bcls,
                                 scale=1.0)
            nc.scalar.dma_start(out=out[:, :], in_=scores)

    nc.compile()
    return nc


def _pack_inputs(x, w_enc, b_enc, w_att, b_att, w_score, w_cls, b_cls):
    """Host-side packing: per-core input maps for run_bass_kernel_spmd."""
    import ml_dtypes

    f8 = ml_dtypes.float8_e4m3
    bf16 = ml_dtypes.bfloat16

    # x [N_INST, D_IN] -> [core, bag, p, chunk, inst] fp8
    xp = x.reshape(N_CORES, BAGS_PER_CORE, INST_PER_BAG, DIN_CHUNKS, 128)
    xp = np.ascontiguousarray(xp.transpose(0, 1, 4, 3, 2)).astype(f8)

    # w_enc [D_IN, D_EMB] * 64 -> [p, chunk, emb] fp8
    wq = (w_enc * W_SCALE).reshape(DIN_CHUNKS, 128, D_EMB).transpose(1, 0, 2)
    wq = np.ascontiguousarray(wq).astype(f8)

    # bf16 pack: w_att | w_score (col, zero-padded) | w_cls
    wscore_col = np.zeros((D_EMB, 1), dtype=np.float32)
    wscore_col[:D_ATT, 0] = w_score
    wbp = np.concatenate(
        [w_att, wscore_col, w_cls], axis=1).astype(bf16)

    # f32 pack: b_enc | b_att (padded) | b_cls (padded)
    wfp = np.zeros((D_EMB, 3), dtype=np.float32)
    wfp[:, 0] = b_enc
    wfp[:D_ATT, 1] = b_att
    wfp[:N_CLS, 2] = b_cls

    shared = {"wq": wq, "wb": wbp, "wf": wfp}
    return [{"xt": xp[c], **shared} for c in range(N_CORES)]


def _numpy_fallback(x, seg, w_enc, b_enc, w_att, b_att, w_score, b_score,
                    w_cls, b_cls):
    emb = np.maximum(x @ w_enc + b_enc, 0.0)
    a = np.tanh(emb @ w_att + b_att)
    logits = a @ w_score + b_score[0]
    out = np.zeros((N_BAGS, N_CLS), dtype=np.float32)
    for bag in range(N_BAGS):
        mask = seg == bag
        lg = logits[mask]
        e = np.exp(lg - lg.max())
        attn = e / e.sum()
        bag_emb = attn @ emb[mask]
        out[bag] = bag_emb @ w_cls + b_cls
    return out


def kernel(**inputs):
    from concourse.bass_utils import run_bass_kernel_spmd

    x = np.asarray(inputs["x"], dtype=np.float32)
    seg = np.asarray(inputs["seg"], dtype=np.int32)
    w_enc = np.asarray(inputs["w_enc"], dtype=np.float32)
    b_enc = np.asarray(inputs["b_enc"], dtype=np.float32)
    w_att = np.asarray(inputs["w_att"], dtype=np.float32)
    b_att = np.asarray(inputs["b_att"], dtype=np.float32)
    w_score = np.asarray(inputs["w_score"], dtype=np.float32)
    b_score = np.asarray(inputs["b_score"], dtype=np.float32)
    w_cls = np.asarray(inputs["w_cls"], dtype=np.float32)
    b_cls = np.asarray(inputs["b_cls"], dtype=np.float32)

    expected_seg = np.repeat(np.arange(N_BAGS, dtype=np.int32), INST_PER_BAG)
    if not np.array_equal(seg, expected_seg):
        # Layout differs from the balanced bags this kernel is built for.
        return _numpy_fallback(x, seg, w_enc, b_enc, w_att, b_att, w_score,
                               b_score, w_cls, b_cls)

    if "nc" not in _CACHE:
        _CACHE["nc"] = _build()
    nc = _CACHE["nc"]

    in_maps = _pack_inputs(x, w_enc, b_enc, w_att, b_att, w_score, w_cls,
                           b_cls)
    res = run_bass_kernel_spmd(nc, in_maps, core_ids=list(range(N_CORES)))
    return np.concatenate(
        [res.results[c]["out"].T for c in range(N_CORES)], axis=0)
